# revision 30
# baseline (speedup 1.0000x reference)
"""Trainium2 Bass kernel for nn_BasicLayer (Erwin-style NSA-MSA sparse ball attention).

Strategy (8 NeuronCores, data-parallel over points):
  - kernel() receives FULL inputs. Each core gets the full x/pos ROTATED so that
    its own 1024 points (16 balls) sit at rows [0:1024]. All computation is
    permutation-equivariant under whole-ball rotation, so one SPMD program works
    for every core with zero device-side differentiation and no collectives.
  - On device (per core): compute xr = rmsnorm(x)*w + rel for ALL 8192 points in
    a ball-per-partition layout [128 balls, 64 pts, 64 dim]; ball-summary keys by
    free-dim group reduction; routing logits for own 1024 points via PE matmuls;
    top-2 ball selection VALUE-based (DVE max8 -> equality masks, no indices);
    sparse gather of selected balls as one-hot mask matmuls on the TensorEngine;
    scores/softmax/weighted-sum on DVE/GPSIMD/ACT; then residual + RMSNorm +
    SwiGLU MLP (PE matmuls) for the own 1024 rows.
"""

import os
import numpy as np

import concourse.bacc as bacc
import concourse.bass as bass
import concourse.mybir as mybir
import concourse.tile as tile
from concourse.masks import make_identity

FP = mybir.dt.float32
N, D = 8192, 64
M = 64          # ball size
NB = N // M     # 128 balls
H, EH = 8, 8
TOPK = 2
NCORES = 8
NPC = N // NCORES   # 1024 points per core
NT = NPC // 128     # 8 point-tiles of 128
BPC = NPC // M      # 16 own balls per core
DH = D * 4          # 256 mlp hidden
EPS = 1.1920929e-07
ISQ8 = float(1.0 / np.sqrt(EH))

A = mybir.AluOpType
AF = mybir.ActivationFunctionType
AX = mybir.AxisListType
FR = mybir.dt.float32r
BH = mybir.dt.bfloat16
FH = mybir.dt.float16


def _fr(ap):
    """View an fp32 AP as float32r (PE runs f32r at 1 cyc/row vs 4 for f32
    when the output free dim is >= 256; rounds inputs to ~19-bit mantissa)."""
    return ap.bitcast(FR)


def _bc(ap, dim, count):
    """Insert a step-0 (broadcast) dim at position `dim` of an AP."""
    new = [list(p) for p in ap.ap]
    new.insert(dim, [0, count])
    return bass.AP(tensor=ap.tensor, offset=ap.offset, ap=new)


def _pf(ap, order):
    """Permute the FREE dims of an AP (order indexes free dims, 0-based)."""
    new = [list(ap.ap[0])] + [list(ap.ap[1 + i]) for i in order]
    return bass.AP(tensor=ap.tensor, offset=ap.offset, ap=new)


def build_kernel_body(nc, tc, ctx, tensors):
    (x_d, pos_d, n1w_d, n2w_d, w1_d, b1_d, w2_d, b2_d, w3_d, b3_d,
     out_d, xr_dram) = tensors

    consts = ctx.enter_context(tc.tile_pool(name="consts", bufs=1))
    big = ctx.enter_context(tc.tile_pool(name="big", bufs=1))
    front_cm = tc.tile_pool(name="front", bufs=1)
    front = front_cm.__enter__()
    ps_tr_cm = tc.tile_pool(name="ps_tr", bufs=2, space="PSUM")
    ps_tr = ps_tr_cm.__enter__()

    ident = consts.tile([128, 128], FP)
    make_identity(nc, ident)
    # f32r variants: values written to these tiles are rounded to f32r at the
    # producer, satisfying the BIR verifier for 1-cyc/row f32r matmuls.

    # ---------------- Stage A: load + xr = rmsnorm(x)*n1w + rel (ball-major) ----
    x_bm = front.tile([128, M, D], FP)       # [ball, m, d]
    pos_bm = front.tile([128, M, D], FP)
    nc.sync.dma_start(out=x_bm, in_=x_d.ap().rearrange("(b m) d -> b m d", m=M))
    nc.sync.dma_start(out=pos_bm, in_=pos_d.ap().rearrange("(b m) d -> b m d", m=M))

    n1w_sb = consts.tile([128, D], FP)
    nc.sync.dma_start(out=n1w_sb,
                      in_=bass.AP(tensor=n1w_d, offset=0, ap=[[0, 128], [1, D]]))

    # ball mean of pos (over m)
    mp8 = front.tile([128, D, 8], FP, tag="mp8")
    nc.vector.tensor_reduce(
        out=mp8, in_=pos_bm.rearrange("b (g m) d -> b d g m", g=8),
        axis=AX.X, op=A.add)
    mp = front.tile([128, D], FP, tag="mp")
    nc.vector.tensor_reduce(out=mp, in_=mp8, axis=AX.X, op=A.add)
    nc.vector.tensor_scalar(mp, mp, 1.0 / M, None, op0=A.mult)

    # rms: 1/sqrt(mean(x^2) + eps)
    sq = front.tile([128, M, D], FP, tag="sq")
    nc.scalar.activation(out=sq, in_=x_bm, func=AF.Square)
    sq8 = front.tile([128, M, 8], FP, tag="sq8")
    nc.vector.tensor_reduce(out=sq8, in_=sq.rearrange("b m (g d) -> b m g d", g=8),
                            axis=AX.X, op=A.add)
    msq = front.tile([128, M], FP, tag="msq")
    nc.vector.tensor_reduce(out=msq, in_=sq8, axis=AX.X, op=A.add)
    nc.vector.tensor_scalar(msq, msq, 1.0 / D, EPS, op0=A.mult, op1=A.add)
    rinv = front.tile([128, M], FP, tag="rinv")
    nc.vector.reciprocal(out=rinv, in_=msq)
    nc.scalar.activation(out=rinv, in_=rinv, func=AF.Sqrt)
    # one Newton step: r <- r*(1.5 - 0.5*msq*r^2)  (ACT Sqrt is low-precision)
    rsqv = front.tile([128, M], FP, tag="rsqv")
    nc.vector.tensor_tensor(out=rsqv, in0=rinv, in1=rinv, op=A.mult)
    nc.vector.tensor_tensor(out=rsqv, in0=rsqv, in1=msq, op=A.mult)
    nc.vector.tensor_scalar(rsqv, rsqv, -0.5, 1.5, op0=A.mult, op1=A.add)
    nc.vector.tensor_tensor(out=rinv, in0=rinv, in1=rsqv, op=A.mult)

    nc.vector.tensor_tensor(out=pos_bm, in0=pos_bm, in1=_bc(mp, 1, M),
                            op=A.subtract)      # pos_bm becomes rel
    xr_bm = front.tile([128, M, D], FP)
    nc.vector.tensor_tensor(out=xr_bm, in0=x_bm, in1=_bc(rinv, 2, D), op=A.mult)
    nc.vector.tensor_tensor(out=xr_bm, in0=xr_bm, in1=_bc(n1w_sb, 1, M), op=A.mult)
    nc.vector.tensor_tensor(out=xr_bm, in0=xr_bm, in1=pos_bm, op=A.add)

    # ---------------- Stage B: KB (gather source) + ball-summary keys ----------
    KB2 = big.tile([128, H, EH, M], BH)    # [ball, h, e, m], bf16 gather source
    nc.vector.tensor_copy(out=KB2, in_=xr_bm.rearrange("b m (h e) -> b h e m", e=EH))
    ks8 = front.tile([128, D, 8], FP, tag="ks8")
    nc.vector.tensor_reduce(out=ks8,
                            in_=xr_bm.rearrange("b (g m) d -> b d g m", g=8),
                            axis=AX.X, op=A.add)
    keys_bm = front.tile([128, D], FP, tag="keys")   # [ball, (h e)] ball-sum (scale-free)
    nc.vector.tensor_reduce(out=keys_bm, in_=ks8, axis=AX.X, op=A.add)
    keysT = front.tile([64, 128], FP)                 # [(h e), ball]
    kt_ps = ps_tr.tile([64, 128], FP, tag="tr")
    nc.tensor.transpose(kt_ps, keys_bm, ident)
    nc.scalar.copy(out=keysT, in_=kt_ps)

    # ---------------- Stage C: own-point layouts --------------------------------
    # xr rows [0:1024] -> DRAM bounce -> point-major + transposed copies
    nc.sync.dma_start(out=xr_dram.ap().rearrange("(b m) d -> b m d", m=M),
                      in_=xr_bm[0:BPC, :, :])
    q_pm = big.tile([128, NT, H, EH], FP)  # per-partition q scalars
    nc.sync.dma_start(out=q_pm,
                      in_=xr_dram.ap().rearrange("(t p) (h e) -> p t h e", p=128, e=EH))
    x_own = big.tile([128, NT, D], FP)
    nc.sync.dma_start(out=x_own,
                      in_=x_d.ap()[0:NPC, :].rearrange("(t p) d -> p t d", p=128))

    qT = front.tile([64, NT, 128], FP)       # [(h e), nt, n128]
    for t in range(NT):
        q_ps = ps_tr.tile([64, 128], FP, tag="tr")
        nc.tensor.transpose(q_ps, q_pm[:, t].rearrange("p h e -> p (h e)"), ident)
        nc.scalar.copy(out=qT[:, t, :], in_=q_ps)

    # hi/lo bf16 split: q.k = qhi.khi + qhi.klo + qlo.khi, each product exact in
    # fp32 -> logits match the fp32 reference to ~1e-7 (PE fp32/transposes round
    # to fp32r, which flips near-tie top-2 selections vs the reference).
    # 4-term split: q.k = qhi.khi + qhi.klo + qlo.khi + qlo.klo — every bf16
    # product is exact in fp32, so logits match the fp32 reference to ~2 ulp.
    # K-row order p = e*4 + j ;  k terms [khi, klo, khi, klo], q [qhi, qhi, qlo, qlo]
    kst4 = front.tile([64, 4, 128], BH)
    qst4 = front.tile([64, 4, NT, 128], BH)
    tmp32 = front.tile([64, NT, 128], FP, tag="tmp32")
    for (src_ap, dst, nfree) in ((keysT, kst4, 1), (qT, qst4, NT)):
        hi = dst[:, 0] if nfree == 1 else dst[:, 0]
        nc.vector.tensor_copy(out=dst[:, 0], in_=src_ap)     # hi (cast bf16)
        t32 = tmp32[:, 0:nfree, :] if nfree == NT else tmp32[:, 0, :]
        nc.vector.tensor_copy(out=t32, in_=dst[:, 0])        # hi back to fp32
        nc.vector.tensor_tensor(out=t32, in0=src_ap, in1=t32, op=A.subtract)
        nc.vector.tensor_copy(out=dst[:, 1], in_=t32)        # lo (cast bf16)
        if nfree == 1:   # k: [khi, klo, khi, klo]
            nc.vector.tensor_copy(out=dst[:, 2], in_=dst[:, 0])
            nc.vector.tensor_copy(out=dst[:, 3], in_=dst[:, 1])
        else:            # q: [qhi, qhi, qlo, qlo] -> reorder: slot1 qhi, slots 2/3 qlo
            nc.vector.tensor_copy(out=dst[:, 2], in_=dst[:, 1])
            nc.vector.tensor_copy(out=dst[:, 3], in_=dst[:, 1])
            nc.vector.tensor_copy(out=dst[:, 1], in_=dst[:, 0])
    kstack = big.tile([32, H, 128], BH)
    qstack = big.tile([32, H, NT, 128], BH)
    for h in range(H):
        nc.sync.dma_start(
            out=kstack[:, h, :],
            in_=bass.AP(tensor=kst4.tensor, offset=kst4.offset + 8 * h * 4 * 128,
                        ap=[[4 * 128, 8], [128, 4], [1, 128]]))
        nc.sync.dma_start(
            out=qstack[:, h],
            in_=bass.AP(tensor=qst4.tensor, offset=qst4.offset + 8 * h * 4 * NT * 128,
                        ap=[[4 * NT * 128, 8], [NT * 128, 4], [128, NT], [1, 128]]))
    front_cm.__exit__(None, None, None)
    ps_tr_cm.__exit__(None, None, None)
    ps_trh_cm = tc.tile_pool(name="ps_trh", bufs=2, space="PSUM")
    ps_trh = ps_trh_cm.__enter__()
    ps_lt_cm = tc.tile_pool(name="ps_lt", bufs=2, space="PSUM")
    ps_lt = ps_lt_cm.__enter__()
    ps_g_cm = tc.tile_pool(name="ps_g", bufs=2, space="PSUM")
    ps_g = ps_g_cm.__enter__()
    work_cm = tc.tile_pool(name="work", bufs=2)
    work = work_cm.__enter__()
    gpool_cm = tc.tile_pool(name="gpool", bufs=2)
    gpool = gpool_cm.__enter__()
    ppool_cm = tc.tile_pool(name="ppool", bufs=1)
    ppool = ppool_cm.__enter__()

    # ---------------- Stage D-H: attention per head -----------------------------
    # All-bf16 data path. Per head:
    #   D: logits lpm[n,b] (exact 4-term bf16 matmul), MAX8 for top-2 values,
    #      point-major one-hot masks via TensorScalarPtr is_equal (2x mode),
    #      bf16 PE transposes to ball-major.
    #   E: gather G = maskT.T @ KB2 per (t, tk); evac PSUM -> bf16 e-major
    #      g_sb[n, e, t, tk, m].
    #   F: qG = g * q (broadcast), tree-sum over e -> scores; ACT Exp per tile
    #      with fp32 accum -> p, Z.
    #   G: prod = g * p (broadcast over e; 2x: both last dims packed),
    #      tree over keys + final TensorReduce -> ws[n, e, t]; scale by 1/Z.
    attn = big.tile([128, NT, H, EH], FP)
    identh = consts.tile([128, 128], BH)
    nc.vector.tensor_copy(out=identh, in_=ident)
    qh_bf = big.tile([128, NT, H, EH], BH)
    nc.vector.tensor_copy(out=qh_bf, in_=q_pm)

    for h in range(H):
        # --- D+E: selection, masks, gather (interleaved per t) ---
        v8 = work.tile([128, NT, 8], FP, tag="v8")
        g_sb = gpool.tile([128, EH, NT, TOPK, M], BH, tag="g")
        for t in range(NT):
            lpm_ps = ps_lt.tile([128, 128], FP, tag="lt")
            nc.tensor.matmul(lpm_ps, lhsT=qstack[:, h, t, :],
                             rhs=kstack[:, h, :], start=True, stop=True)
            nc.vector.max(out=v8[:, t, :], in_=lpm_ps)
            mask_pm = work.tile([128, TOPK, 128], BH, tag="mpm")
            mt_ps = ps_trh.tile([128, TOPK, 128], BH, tag="trh")
            for tk in range(TOPK):
                nc.vector.tensor_scalar(mask_pm[:, tk], lpm_ps,
                                        v8[:, t, tk:tk + 1], None,
                                        op0=A.is_equal)
                nc.tensor.transpose(mt_ps[:, tk], mask_pm[:, tk], identh)
            maskT = work.tile([128, TOPK, 128], BH, tag="mT")
            nc.scalar.copy(out=maskT, in_=mt_ps)
            g_ps = ps_g.tile([128, TOPK, 512], FP, tag="g")
            for tk in range(TOPK):
                nc.tensor.matmul(g_ps[:, tk], lhsT=maskT[:, tk],
                                 rhs=KB2[:, h].rearrange("b e m -> b (e m)"),
                                 start=True, stop=True)
            # one evac per t: PSUM (tk, e, m) viewed as (e, tk, m)
            nc.scalar.copy(out=g_sb[:, :, t, :, :],
                           in_=_pf(g_ps.rearrange("p k (e m) -> p k e m", m=M),
                                   [1, 0, 2]))

        # --- F: scores via qG (fp16) + e-tree; exp with accum ---
        # (ISA allows <=3 free dims: keep views as (e, t, (tk m)) or flatter.)
        # qG split across engines: e 0:4 on DVE, e 4:8 on GPSIMD (both ~equal
        # wall; GPSIMD is otherwise idle).
        q_lo = _bc(_pf(qh_bf[:, :, h, 0:4], [1, 0]), 3, TOPK * M)
        q_hi = _bc(_pf(qh_bf[:, :, h, 4:8], [1, 0]), 3, TOPK * M)
        qlo = work.tile([128, 4, NT, TOPK * M], FH, tag="qlo")
        qhi = work.tile([128, 4, NT, TOPK * M], FH, tag="qhi")
        nc.vector.tensor_tensor(
            out=qlo, in0=g_sb[:, 0:4].rearrange("p e t k m -> p e t (k m)"),
            in1=q_lo, op=A.mult)
        nc.gpsimd.tensor_tensor(
            out=qhi, in0=g_sb[:, 4:8].rearrange("p e t k m -> p e t (k m)"),
            in1=q_hi, op=A.mult)
        s01 = work.tile([128, 4, NT * TOPK * M], FH, tag="s01")
        nc.vector.tensor_tensor(out=s01,
                                in0=qlo.rearrange("p e t j -> p e (t j)"),
                                in1=qhi.rearrange("p e t j -> p e (t j)"),
                                op=A.add)
        s2 = work.tile([128, 2, NT * TOPK * M], FH, tag="s2")
        nc.vector.tensor_tensor(out=s2, in0=s01[:, 0:2], in1=s01[:, 2:4],
                                op=A.add)
        s_sb = work.tile([128, NT, TOPK, M], FH, tag="s")
        nc.vector.tensor_tensor(out=s_sb.rearrange("p t k m -> p (t k m)"),
                                in0=s2[:, 0], in1=s2[:, 1], op=A.add)
        p_sb = work.tile([128, NT, TOPK, M], BH, tag="p")
        zh = work.tile([128, NT], FP, tag="z")
        for t in range(NT):
            nc.scalar.activation(out=p_sb[:, t], in_=s_sb[:, t], func=AF.Exp,
                                 scale=ISQ8, accum_out=zh[:, t:t + 1])
        nc.vector.reciprocal(out=zh, in_=zh)

        # --- G: weighted sum ---
        g_v = g_sb.rearrange("p e t k m -> p e t (k m)")
        prod = ppool.tile([128, EH, NT, TOPK * M], BH, tag="prod")
        p_in1 = _bc(p_sb.rearrange("p t k m -> p t (k m)"), 1, EH)
        nc.vector.tensor_tensor(out=prod, in0=g_v, in1=p_in1, op=A.mult)
        pr = prod
        w64 = work.tile([128, EH, NT, 64], BH, tag="w64")
        nc.vector.tensor_tensor(out=w64, in0=pr[:, :, :, 0:64],
                                in1=pr[:, :, :, 64:128], op=A.add)
        w32 = work.tile([128, EH, NT, 32], BH, tag="w32")
        nc.vector.tensor_tensor(out=w32, in0=w64[:, :, :, 0:32],
                                in1=w64[:, :, :, 32:64], op=A.add)
        w16 = work.tile([128, EH, NT, 16], BH, tag="w16")
        nc.vector.tensor_tensor(out=w16, in0=w32[:, :, :, 0:16],
                                in1=w32[:, :, :, 16:32], op=A.add)
        w8 = work.tile([128, EH, NT, 8], BH, tag="w8")
        nc.vector.tensor_tensor(out=w8, in0=w16[:, :, :, 0:8],
                                in1=w16[:, :, :, 8:16], op=A.add)
        ws = work.tile([128, EH, NT], FP, tag="ws")
        nc.vector.tensor_reduce(out=ws, in_=w8, axis=AX.X, op=A.add)
        # ws[n, e, t] viewed as (t, e) to write attn[n, t, h, e]
        nc.vector.tensor_tensor(out=attn[:, :, h, :], in0=_pf(ws[:, :, :], [1, 0]),
                                in1=_bc(zh, 2, EH), op=A.mult)

    # ---------------- Stage I: residual + rmsnorm2 + SwiGLU MLP ----------------
    ppool_cm.__exit__(None, None, None)
    gpool_cm.__exit__(None, None, None)
    work_cm.__exit__(None, None, None)
    ps_g_cm.__exit__(None, None, None)
    ps_lt_cm.__exit__(None, None, None)
    ps_trh_cm.__exit__(None, None, None)
    ps_tr = ctx.enter_context(tc.tile_pool(name="ps_tr2", bufs=2, space="PSUM"))
    mlpw = ctx.enter_context(tc.tile_pool(name="mlpw", bufs=1))
    y = mlpw.tile([128, NT, D], FP)
    nc.vector.tensor_tensor(out=y, in0=x_own,
                            in1=attn.rearrange("p t h e -> p t (h e)"), op=A.add)

    n2w_sb = consts.tile([128, D], FP)
    nc.sync.dma_start(out=n2w_sb,
                      in_=bass.AP(tensor=n2w_d, offset=0, ap=[[0, 128], [1, D]]))
    sq2 = mlpw.tile([128, NT, D], FP, tag="sq2")
    nc.scalar.activation(out=sq2, in_=y, func=AF.Square)
    ri2 = mlpw.tile([128, NT], FP, tag="ri2")
    nc.vector.tensor_reduce(out=ri2, in_=sq2, axis=AX.X, op=A.add)
    nc.vector.tensor_scalar(ri2, ri2, 1.0 / D, EPS, op0=A.mult, op1=A.add)
    nc.vector.reciprocal(out=ri2, in_=ri2)
    nc.scalar.activation(out=ri2, in_=ri2, func=AF.Sqrt)
    x2 = mlpw.tile([128, NT, D], FP)
    nc.vector.tensor_tensor(out=x2, in0=y, in1=_bc(ri2, 2, D), op=A.mult)
    nc.vector.tensor_tensor(out=x2, in0=x2, in1=_bc(n2w_sb, 1, NT), op=A.mult)

    # weights: w1/w2 [256, 64] -> transposed [64, 256]; w3 [64, 256] -> [256, 64]
    w1n = consts.tile([128, 2, D], FP)
    w2n = consts.tile([128, 2, D], FP)
    w3n = consts.tile([64, DH], FP)
    nc.sync.dma_start(out=w1n, in_=w1_d.ap().rearrange("(c j) d -> j c d", j=128))
    nc.sync.dma_start(out=w2n, in_=w2_d.ap().rearrange("(c j) d -> j c d", j=128))
    nc.sync.dma_start(out=w3n, in_=w3_d.ap())
    w1T = consts.tile([64, DH], FR)   # [d, j]
    w2T = consts.tile([64, DH], FR)
    w3T = consts.tile([128, 2, D], FP)  # [j128, c, e]
    for c in range(2):
        t_ps = ps_tr.tile([64, 128], FP, tag="tr")
        nc.tensor.transpose(t_ps, w1n[:, c, :], ident)
        nc.scalar.copy(out=w1T[:, 128 * c:128 * (c + 1)], in_=t_ps)
        t_ps2 = ps_tr.tile([64, 128], FP, tag="tr")
        nc.tensor.transpose(t_ps2, w2n[:, c, :], ident)
        nc.scalar.copy(out=w2T[:, 128 * c:128 * (c + 1)], in_=t_ps2)
        t_ps3 = ps_tr.tile([128, 64], FP, tag="tr")
        nc.tensor.transpose(t_ps3, w3n[:, 128 * c:128 * (c + 1)],
                            ident[0:64, 0:64])
        nc.scalar.copy(out=w3T[:, c, :], in_=t_ps3)
    b1_sb = consts.tile([128, 2], FP)
    b2_sb = consts.tile([128, 2], FP)
    nc.sync.dma_start(out=b1_sb, in_=b1_d.ap().rearrange("(c j) -> j c", j=128))
    nc.sync.dma_start(out=b2_sb, in_=b2_d.ap().rearrange("(c j) -> j c", j=128))
    b3_sb = consts.tile([128, D], FP)
    nc.sync.dma_start(out=b3_sb,
                      in_=bass.AP(tensor=b3_d, offset=0, ap=[[0, 128], [1, D]]))

    # x2T [d, n] via PE transposes
    x2T = mlpw.tile([64, NT, 128], FR)
    for t in range(NT):
        xt_ps = ps_tr.tile([64, 128], FP, tag="tr")
        nc.tensor.transpose(xt_ps, x2[:, t, :], ident)
        nc.scalar.copy(out=x2T[:, t, :], in_=xt_ps)

    # y3 = y + b3 (residual with bias folded)
    nc.vector.tensor_tensor(out=y, in0=y, in1=_bc(b3_sb, 1, NT), op=A.add)

    # h1T/h2T [j, n] = w @ x2T; silu/bias per-partition
    hhT = mlpw.tile([128, 2, NT, 128], FP)   # [j128, c, nt, n]
    ps_m = ctx.enter_context(tc.tile_pool(name="ps_m", bufs=1, space="PSUM"))
    for c in range(2):
        h1_ps = ps_m.tile([128, NT, 128], FP, tag="h1")
        h2_ps = ps_m.tile([128, NT, 128], FP, tag="h2")
        for half in range(2):
            sl = slice(4 * half, 4 * (half + 1))
            nc.tensor.matmul(h1_ps[:, sl, :].rearrange("j t n -> j (t n)"),
                             lhsT=_fr(w1T[:, 128 * c:128 * (c + 1)]),
                             rhs=_fr(x2T[:, sl, :].rearrange("d t n -> d (t n)")),
                             start=True, stop=True)
            nc.tensor.matmul(h2_ps[:, sl, :].rearrange("j t n -> j (t n)"),
                             lhsT=_fr(w2T[:, 128 * c:128 * (c + 1)]),
                             rhs=_fr(x2T[:, sl, :].rearrange("d t n -> d (t n)")),
                             start=True, stop=True)
        h1b = mlpw.tile([128, NT, 128], FP, tag="h1b")
        nc.vector.tensor_scalar(h1b, h1_ps, b1_sb[:, c:c + 1], None, op0=A.add)
        sgm = mlpw.tile([128, NT, 128], FP, tag="sgm")
        nc.scalar.activation(out=sgm, in_=h1b, func=AF.Sigmoid)
        h2s = mlpw.tile([128, NT, 128], FP, tag="h2s")
        nc.vector.tensor_scalar(h2s, h2_ps, b2_sb[:, c:c + 1], None, op0=A.add)
        nc.vector.tensor_tensor(out=h1b, in0=h1b, in1=sgm, op=A.mult)
        nc.vector.tensor_tensor(out=hhT[:, c], in0=h1b, in1=h2s, op=A.mult)

    final = mlpw.tile([128, NT, D], FP)
    for t in range(NT):
        o_ps = ps_m.tile([128, D], FP, tag="o")
        for c in range(2):
            nc.tensor.matmul(o_ps, lhsT=hhT[:, c, t, :], rhs=w3T[:, c, :],
                             start=(c == 0), stop=(c == 1))
        nc.vector.scalar_tensor_tensor(out=final[:, t], in0=o_ps, scalar=1.0,
                                       in1=y[:, t], op0=A.mult, op1=A.add)

    nc.sync.dma_start(out=out_d.ap().rearrange("(t p) d -> p t d", p=128),
                      in_=final)


def build_program():
    from contextlib import ExitStack
    nc = bacc.Bacc("TRN2", target_bir_lowering=False, debug=False,
                   num_devices=NCORES)
    x_d = nc.dram_tensor("x", [N, D], FP, kind="ExternalInput")
    pos_d = nc.dram_tensor("pos", [N, D], FP, kind="ExternalInput")
    n1w_d = nc.dram_tensor("norm1_w", [D], FP, kind="ExternalInput")
    n2w_d = nc.dram_tensor("norm2_w", [D], FP, kind="ExternalInput")
    w1_d = nc.dram_tensor("w1_w", [DH, D], FP, kind="ExternalInput")
    b1_d = nc.dram_tensor("w1_b", [DH], FP, kind="ExternalInput")
    w2_d = nc.dram_tensor("w2_w", [DH, D], FP, kind="ExternalInput")
    b2_d = nc.dram_tensor("w2_b", [DH], FP, kind="ExternalInput")
    w3_d = nc.dram_tensor("w3_w", [D, DH], FP, kind="ExternalInput")
    b3_d = nc.dram_tensor("w3_b", [D], FP, kind="ExternalInput")
    out_d = nc.dram_tensor("out", [NPC, D], FP, kind="ExternalOutput")
    xr_dram = nc.dram_tensor("xr_own", [NPC, D], FP)

    tensors = (x_d, pos_d, n1w_d, n2w_d, w1_d, b1_d, w2_d, b2_d, w3_d, b3_d,
               out_d, xr_dram)
    with tile.TileContext(nc) as tc:
        with ExitStack() as ctx:
            build_kernel_body(nc, tc, ctx, tensors)
    nc.compile()
    return nc


_NC_CACHE = None


def _get_nc():
    global _NC_CACHE
    if _NC_CACHE is None:
        _NC_CACHE = build_program()
    return _NC_CACHE


def make_in_maps(inputs):
    in_maps = []
    for c in range(NCORES):
        sh = c * NPC
        m = {
            "x": np.ascontiguousarray(np.roll(inputs["x"], -sh, axis=0)),
            "pos": np.ascontiguousarray(np.roll(inputs["pos"], -sh, axis=0)),
            "norm1_w": inputs["norm1_w"], "norm2_w": inputs["norm2_w"],
            "w1_w": inputs["w1_w"], "w1_b": inputs["w1_b"],
            "w2_w": inputs["w2_w"], "w2_b": inputs["w2_b"],
            "w3_w": inputs["w3_w"], "w3_b": inputs["w3_b"],
        }
        in_maps.append({k: np.asarray(v, dtype=np.float32) for k, v in m.items()})
    return in_maps


def run_on_hw(inputs, trace=False):
    from concourse.bass_utils import run_bass_kernel_spmd
    nc = _get_nc()
    res = run_bass_kernel_spmd(nc, make_in_maps(inputs), list(range(NCORES)),
                               trace=trace)
    out = np.concatenate([res.results[c]["out"] for c in range(NCORES)], axis=0)
    return out, res


def kernel(**inputs):
    out, _ = run_on_hw(inputs, trace=False)
    return out



# revision 33
# speedup vs baseline: 1.1388x; 1.1388x over previous
"""Trainium2 Bass kernel for nn_BasicLayer (Erwin-style NSA-MSA sparse ball attention).

Strategy (8 NeuronCores, data-parallel over points):
  - kernel() receives FULL inputs. Each core gets the full x/pos ROTATED so that
    its own 1024 points (16 balls) sit at rows [0:1024]. All computation is
    permutation-equivariant under whole-ball rotation, so one SPMD program works
    for every core with zero device-side differentiation and no collectives.
  - On device (per core): compute xr = rmsnorm(x)*w + rel for ALL 8192 points in
    a ball-per-partition layout [128 balls, 64 pts, 64 dim]; ball-summary keys by
    free-dim group reduction; routing logits for own 1024 points via PE matmuls;
    top-2 ball selection VALUE-based (DVE max8 -> equality masks, no indices);
    sparse gather of selected balls as one-hot mask matmuls on the TensorEngine;
    scores/softmax/weighted-sum on DVE/GPSIMD/ACT; then residual + RMSNorm +
    SwiGLU MLP (PE matmuls) for the own 1024 rows.
"""

import os
import numpy as np

import concourse.bacc as bacc
import concourse.bass as bass
import concourse.mybir as mybir
import concourse.tile as tile
from concourse.masks import make_identity

FP = mybir.dt.float32
N, D = 8192, 64
M = 64          # ball size
NB = N // M     # 128 balls
H, EH = 8, 8
TOPK = 2
NCORES = 8
NPC = N // NCORES   # 1024 points per core
NT = NPC // 128     # 8 point-tiles of 128
BPC = NPC // M      # 16 own balls per core
DH = D * 4          # 256 mlp hidden
EPS = 1.1920929e-07
ISQ8 = float(1.0 / np.sqrt(EH))

A = mybir.AluOpType
AF = mybir.ActivationFunctionType
AX = mybir.AxisListType
FR = mybir.dt.float32r
BH = mybir.dt.bfloat16
FH = mybir.dt.float16


def _fr(ap):
    """View an fp32 AP as float32r (PE runs f32r at 1 cyc/row vs 4 for f32
    when the output free dim is >= 256; rounds inputs to ~19-bit mantissa)."""
    return ap.bitcast(FR)


def _bc(ap, dim, count):
    """Insert a step-0 (broadcast) dim at position `dim` of an AP."""
    new = [list(p) for p in ap.ap]
    new.insert(dim, [0, count])
    return bass.AP(tensor=ap.tensor, offset=ap.offset, ap=new)


def _pf(ap, order):
    """Permute the FREE dims of an AP (order indexes free dims, 0-based)."""
    new = [list(ap.ap[0])] + [list(ap.ap[1 + i]) for i in order]
    return bass.AP(tensor=ap.tensor, offset=ap.offset, ap=new)


def build_kernel_body(nc, tc, ctx, tensors):
    (x_d, pos_d, n1w_d, n2w_d, w1_d, b1_d, w2_d, b2_d, w3_d, b3_d,
     out_d, xr_dram) = tensors

    consts = ctx.enter_context(tc.tile_pool(name="consts", bufs=1))
    big = ctx.enter_context(tc.tile_pool(name="big", bufs=1))
    front_cm = tc.tile_pool(name="front", bufs=1)
    front = front_cm.__enter__()
    ps_tr_cm = tc.tile_pool(name="ps_tr", bufs=2, space="PSUM")
    ps_tr = ps_tr_cm.__enter__()

    ident = consts.tile([128, 128], FP)
    make_identity(nc, ident)
    # f32r variants: values written to these tiles are rounded to f32r at the
    # producer, satisfying the BIR verifier for 1-cyc/row f32r matmuls.

    # ---------------- Stage A: load + xr = rmsnorm(x)*n1w + rel (ball-major) ----
    x_bm = front.tile([128, M, D], FP)       # [ball, m, d]
    pos_bm = front.tile([128, M, D], FP)
    nc.sync.dma_start(out=x_bm, in_=x_d.ap().rearrange("(b m) d -> b m d", m=M))
    nc.sync.dma_start(out=pos_bm, in_=pos_d.ap().rearrange("(b m) d -> b m d", m=M))

    n1w_sb = consts.tile([128, D], FP)
    nc.sync.dma_start(out=n1w_sb,
                      in_=bass.AP(tensor=n1w_d, offset=0, ap=[[0, 128], [1, D]]))

    # ball mean of pos (over m)
    mp8 = front.tile([128, D, 8], FP, tag="mp8")
    nc.vector.tensor_reduce(
        out=mp8, in_=pos_bm.rearrange("b (g m) d -> b d g m", g=8),
        axis=AX.X, op=A.add)
    mp = front.tile([128, D], FP, tag="mp")
    nc.vector.tensor_reduce(out=mp, in_=mp8, axis=AX.X, op=A.add)
    nc.vector.tensor_scalar(mp, mp, 1.0 / M, None, op0=A.mult)

    # rms: 1/sqrt(mean(x^2) + eps)
    sq = front.tile([128, M, D], FP, tag="sq")
    nc.scalar.activation(out=sq, in_=x_bm, func=AF.Square)
    sq8 = front.tile([128, M, 8], FP, tag="sq8")
    nc.vector.tensor_reduce(out=sq8, in_=sq.rearrange("b m (g d) -> b m g d", g=8),
                            axis=AX.X, op=A.add)
    msq = front.tile([128, M], FP, tag="msq")
    nc.vector.tensor_reduce(out=msq, in_=sq8, axis=AX.X, op=A.add)
    nc.vector.tensor_scalar(msq, msq, 1.0 / D, EPS, op0=A.mult, op1=A.add)
    rinv = front.tile([128, M], FP, tag="rinv")
    nc.vector.reciprocal(out=rinv, in_=msq)
    nc.scalar.activation(out=rinv, in_=rinv, func=AF.Sqrt)
    # one Newton step: r <- r*(1.5 - 0.5*msq*r^2)  (ACT Sqrt is low-precision)
    rsqv = front.tile([128, M], FP, tag="rsqv")
    nc.vector.tensor_tensor(out=rsqv, in0=rinv, in1=rinv, op=A.mult)
    nc.vector.tensor_tensor(out=rsqv, in0=rsqv, in1=msq, op=A.mult)
    nc.vector.tensor_scalar(rsqv, rsqv, -0.5, 1.5, op0=A.mult, op1=A.add)
    nc.vector.tensor_tensor(out=rinv, in0=rinv, in1=rsqv, op=A.mult)

    nc.vector.tensor_tensor(out=pos_bm, in0=pos_bm, in1=_bc(mp, 1, M),
                            op=A.subtract)      # pos_bm becomes rel
    xr_bm = front.tile([128, M, D], FP)
    nc.vector.tensor_tensor(out=xr_bm, in0=x_bm, in1=_bc(rinv, 2, D), op=A.mult)
    nc.vector.tensor_tensor(out=xr_bm, in0=xr_bm, in1=_bc(n1w_sb, 1, M), op=A.mult)
    nc.vector.tensor_tensor(out=xr_bm, in0=xr_bm, in1=pos_bm, op=A.add)

    # ---------------- Stage B: KB (gather source) + ball-summary keys ----------
    KB2 = big.tile([128, H, EH, M], BH)    # [ball, h, e, m], bf16 gather source
    nc.vector.tensor_copy(out=KB2, in_=xr_bm.rearrange("b m (h e) -> b h e m", e=EH))
    ks8 = front.tile([128, D, 8], FP, tag="ks8")
    nc.vector.tensor_reduce(out=ks8,
                            in_=xr_bm.rearrange("b (g m) d -> b d g m", g=8),
                            axis=AX.X, op=A.add)
    keys_bm = front.tile([128, D], FP, tag="keys")   # [ball, (h e)] ball-sum (scale-free)
    nc.vector.tensor_reduce(out=keys_bm, in_=ks8, axis=AX.X, op=A.add)
    keysT = front.tile([64, 128], FP)                 # [(h e), ball]
    kt_ps = ps_tr.tile([64, 128], FP, tag="tr")
    nc.tensor.transpose(kt_ps, keys_bm, ident)
    nc.scalar.copy(out=keysT, in_=kt_ps)

    # ---------------- Stage C: own-point layouts --------------------------------
    # xr rows [0:1024] -> DRAM bounce -> point-major + transposed copies
    nc.sync.dma_start(out=xr_dram.ap().rearrange("(b m) d -> b m d", m=M),
                      in_=xr_bm[0:BPC, :, :])
    q_pm = big.tile([128, NT, H, EH], FP)  # per-partition q scalars
    nc.sync.dma_start(out=q_pm,
                      in_=xr_dram.ap().rearrange("(t p) (h e) -> p t h e", p=128, e=EH))
    x_own = big.tile([128, NT, D], FP)
    nc.sync.dma_start(out=x_own,
                      in_=x_d.ap()[0:NPC, :].rearrange("(t p) d -> p t d", p=128))

    qT = front.tile([64, NT, 128], FP)       # [(h e), nt, n128]
    for t in range(NT):
        q_ps = ps_tr.tile([64, 128], FP, tag="tr")
        nc.tensor.transpose(q_ps, q_pm[:, t].rearrange("p h e -> p (h e)"), ident)
        nc.scalar.copy(out=qT[:, t, :], in_=q_ps)

    # hi/lo bf16 split: q.k = qhi.khi + qhi.klo + qlo.khi, each product exact in
    # fp32 -> logits match the fp32 reference to ~1e-7 (PE fp32/transposes round
    # to fp32r, which flips near-tie top-2 selections vs the reference).
    # 4-term split: q.k = qhi.khi + qhi.klo + qlo.khi + qlo.klo — every bf16
    # product is exact in fp32, so logits match the fp32 reference to ~2 ulp.
    # K-row order p = e*4 + j ;  k terms [khi, klo, khi, klo], q [qhi, qhi, qlo, qlo]
    kst4 = front.tile([64, 4, 128], BH)
    qst4 = front.tile([64, 4, NT, 128], BH)
    tmp32 = front.tile([64, NT, 128], FP, tag="tmp32")
    for (src_ap, dst, nfree) in ((keysT, kst4, 1), (qT, qst4, NT)):
        hi = dst[:, 0] if nfree == 1 else dst[:, 0]
        nc.vector.tensor_copy(out=dst[:, 0], in_=src_ap)     # hi (cast bf16)
        t32 = tmp32[:, 0:nfree, :] if nfree == NT else tmp32[:, 0, :]
        nc.vector.tensor_copy(out=t32, in_=dst[:, 0])        # hi back to fp32
        nc.vector.tensor_tensor(out=t32, in0=src_ap, in1=t32, op=A.subtract)
        nc.vector.tensor_copy(out=dst[:, 1], in_=t32)        # lo (cast bf16)
        if nfree == 1:   # k: [khi, klo, khi, klo]
            nc.vector.tensor_copy(out=dst[:, 2], in_=dst[:, 0])
            nc.vector.tensor_copy(out=dst[:, 3], in_=dst[:, 1])
        else:            # q: [qhi, qhi, qlo, qlo] -> reorder: slot1 qhi, slots 2/3 qlo
            nc.vector.tensor_copy(out=dst[:, 2], in_=dst[:, 1])
            nc.vector.tensor_copy(out=dst[:, 3], in_=dst[:, 1])
            nc.vector.tensor_copy(out=dst[:, 1], in_=dst[:, 0])
    kstack = big.tile([32, H, 128], BH)
    qstack = big.tile([32, H, NT, 128], BH)
    for h in range(H):
        nc.sync.dma_start(
            out=kstack[:, h, :],
            in_=bass.AP(tensor=kst4.tensor, offset=kst4.offset + 8 * h * 4 * 128,
                        ap=[[4 * 128, 8], [128, 4], [1, 128]]))
        nc.sync.dma_start(
            out=qstack[:, h],
            in_=bass.AP(tensor=qst4.tensor, offset=qst4.offset + 8 * h * 4 * NT * 128,
                        ap=[[4 * NT * 128, 8], [NT * 128, 4], [128, NT], [1, 128]]))
    front_cm.__exit__(None, None, None)
    ps_tr_cm.__exit__(None, None, None)
    ps_trh_cm = tc.tile_pool(name="ps_trh", bufs=2, space="PSUM")
    ps_trh = ps_trh_cm.__enter__()
    ps_lt_cm = tc.tile_pool(name="ps_lt", bufs=2, space="PSUM")
    ps_lt = ps_lt_cm.__enter__()
    ps_g_cm = tc.tile_pool(name="ps_g", bufs=2, space="PSUM")
    ps_g = ps_g_cm.__enter__()
    work_cm = tc.tile_pool(name="work", bufs=2)
    work = work_cm.__enter__()
    gpool_cm = tc.tile_pool(name="gpool", bufs=2)
    gpool = gpool_cm.__enter__()
    ppool_cm = tc.tile_pool(name="ppool", bufs=2)
    ppool = ppool_cm.__enter__()

    # ---------------- Stage D-H: attention per head -----------------------------
    # All-bf16 data path. Per head:
    #   D: logits lpm[n,b] (exact 4-term bf16 matmul), MAX8 for top-2 values,
    #      point-major one-hot masks via TensorScalarPtr is_equal (2x mode),
    #      bf16 PE transposes to ball-major.
    #   E: gather G = maskT.T @ KB2 per (t, tk); evac PSUM -> bf16 e-major
    #      g_sb[n, e, t, tk, m].
    #   F: qG = g * q (broadcast), tree-sum over e -> scores; ACT Exp per tile
    #      with fp32 accum -> p, Z.
    #   G: prod = g * p (broadcast over e; 2x: both last dims packed),
    #      tree over keys + final TensorReduce -> ws[n, e, t]; scale by 1/Z.
    attn = big.tile([128, NT, H, EH], FP)
    identh = consts.tile([128, 128], BH)
    nc.vector.tensor_copy(out=identh, in_=ident)
    qh_bf = big.tile([128, NT, H, EH], BH)
    nc.vector.tensor_copy(out=qh_bf, in_=q_pm)

    for h in range(H):
        # --- D+E: selection, masks, gather (interleaved per t) ---
        v8 = work.tile([128, NT, 8], FP, tag="v8")
        g_sb = gpool.tile([128, EH, NT, TOPK, M], BH, tag="g")
        for t in range(NT):
            lpm_ps = ps_lt.tile([128, 128], FP, tag="lt")
            nc.tensor.matmul(lpm_ps, lhsT=qstack[:, h, t, :],
                             rhs=kstack[:, h, :], start=True, stop=True)
            nc.vector.max(out=v8[:, t, :], in_=lpm_ps)
            lpm_sb = work.tile([128, 128], FP, tag="lpm")
            nc.scalar.copy(out=lpm_sb, in_=lpm_ps)
            mask_pm = work.tile([128, TOPK, 128], BH, tag="mpm")
            mt_ps = ps_trh.tile([128, TOPK, 128], BH, tag="trh")
            for tk in range(TOPK):
                nc.vector.tensor_scalar(mask_pm[:, tk], lpm_sb,
                                        v8[:, t, tk:tk + 1], None,
                                        op0=A.is_equal)
                nc.tensor.transpose(mt_ps[:, tk], mask_pm[:, tk], identh)
            maskT = work.tile([128, TOPK, 128], BH, tag="mT")
            nc.scalar.copy(out=maskT, in_=mt_ps)
            g_ps = ps_g.tile([128, TOPK, 512], FP, tag="g")
            for tk in range(TOPK):
                nc.tensor.matmul(g_ps[:, tk], lhsT=maskT[:, tk],
                                 rhs=KB2[:, h].rearrange("b e m -> b (e m)"),
                                 start=True, stop=True)
            # one evac per t: PSUM (tk, e, m) viewed as (e, tk, m)
            nc.scalar.copy(out=g_sb[:, :, t, :, :],
                           in_=_pf(g_ps.rearrange("p k (e m) -> p k e m", m=M),
                                   [1, 0, 2]))

        # --- F: scores via qG (fp16) + e-tree; exp with accum ---
        # (ISA allows <=3 free dims: keep views as (e, t, (tk m)) or flatter.)
        # qG split across engines: e 0:4 on DVE, e 4:8 on GPSIMD (both ~equal
        # wall; GPSIMD is otherwise idle).
        q_in1 = _bc(_pf(qh_bf[:, :, h, :], [1, 0]), 3, TOPK * M)
        qG = work.tile([128, EH, NT, TOPK * M], FH, tag="qG")
        nc.vector.tensor_tensor(
            out=qG, in0=g_sb.rearrange("p e t k m -> p e t (k m)"),
            in1=q_in1, op=A.mult)
        qG_f = qG.rearrange("p e t j -> p e (t j)")
        s01 = work.tile([128, 4, NT * TOPK * M], FH, tag="s01")
        nc.vector.tensor_tensor(out=s01, in0=qG_f[:, 0:4], in1=qG_f[:, 4:8],
                                op=A.add)
        s2 = work.tile([128, 2, NT * TOPK * M], FH, tag="s2")
        nc.vector.tensor_tensor(out=s2, in0=s01[:, 0:2], in1=s01[:, 2:4],
                                op=A.add)
        s_sb = work.tile([128, NT, TOPK, M], FH, tag="s")
        nc.vector.tensor_tensor(out=s_sb.rearrange("p t k m -> p (t k m)"),
                                in0=s2[:, 0], in1=s2[:, 1], op=A.add)
        p_sb = work.tile([128, NT, TOPK, M], BH, tag="p")
        zh = work.tile([128, NT], FP, tag="z")
        for t in range(NT):
            nc.scalar.activation(out=p_sb[:, t], in_=s_sb[:, t], func=AF.Exp,
                                 scale=ISQ8, accum_out=zh[:, t:t + 1])
        nc.vector.reciprocal(out=zh, in_=zh)

        # --- G: weighted sum ---
        g_v = g_sb.rearrange("p e t k m -> p e t (k m)")
        prod = ppool.tile([128, EH, NT, TOPK * M], BH, tag="prod")
        p_in1 = _bc(p_sb.rearrange("p t k m -> p t (k m)"), 1, EH)
        nc.vector.tensor_tensor(out=prod, in0=g_v, in1=p_in1, op=A.mult)
        pr = prod
        w64 = work.tile([128, EH, NT, 64], BH, tag="w64")
        nc.vector.tensor_tensor(out=w64, in0=pr[:, :, :, 0:64],
                                in1=pr[:, :, :, 64:128], op=A.add)
        w32 = work.tile([128, EH, NT, 32], BH, tag="w32")
        nc.vector.tensor_tensor(out=w32, in0=w64[:, :, :, 0:32],
                                in1=w64[:, :, :, 32:64], op=A.add)
        w16 = work.tile([128, EH, NT, 16], BH, tag="w16")
        nc.vector.tensor_tensor(out=w16, in0=w32[:, :, :, 0:16],
                                in1=w32[:, :, :, 16:32], op=A.add)
        w8 = work.tile([128, EH, NT, 8], BH, tag="w8")
        nc.vector.tensor_tensor(out=w8, in0=w16[:, :, :, 0:8],
                                in1=w16[:, :, :, 8:16], op=A.add)
        ws = work.tile([128, EH, NT], FP, tag="ws")
        nc.vector.tensor_reduce(out=ws, in_=w8, axis=AX.X, op=A.add)
        # ws[n, e, t] viewed as (t, e) to write attn[n, t, h, e]
        nc.vector.tensor_tensor(out=attn[:, :, h, :], in0=_pf(ws[:, :, :], [1, 0]),
                                in1=_bc(zh, 2, EH), op=A.mult)

    # ---------------- Stage I: residual + rmsnorm2 + SwiGLU MLP ----------------
    ppool_cm.__exit__(None, None, None)
    gpool_cm.__exit__(None, None, None)
    work_cm.__exit__(None, None, None)
    ps_g_cm.__exit__(None, None, None)
    ps_lt_cm.__exit__(None, None, None)
    ps_trh_cm.__exit__(None, None, None)
    ps_tr = ctx.enter_context(tc.tile_pool(name="ps_tr2", bufs=2, space="PSUM"))
    mlpw = ctx.enter_context(tc.tile_pool(name="mlpw", bufs=1))
    y = mlpw.tile([128, NT, D], FP)
    nc.vector.tensor_tensor(out=y, in0=x_own,
                            in1=attn.rearrange("p t h e -> p t (h e)"), op=A.add)

    n2w_sb = consts.tile([128, D], FP)
    nc.sync.dma_start(out=n2w_sb,
                      in_=bass.AP(tensor=n2w_d, offset=0, ap=[[0, 128], [1, D]]))
    sq2 = mlpw.tile([128, NT, D], FP, tag="sq2")
    nc.scalar.activation(out=sq2, in_=y, func=AF.Square)
    ri2 = mlpw.tile([128, NT], FP, tag="ri2")
    nc.vector.tensor_reduce(out=ri2, in_=sq2, axis=AX.X, op=A.add)
    nc.vector.tensor_scalar(ri2, ri2, 1.0 / D, EPS, op0=A.mult, op1=A.add)
    nc.vector.reciprocal(out=ri2, in_=ri2)
    nc.scalar.activation(out=ri2, in_=ri2, func=AF.Sqrt)
    x2 = mlpw.tile([128, NT, D], FP)
    nc.vector.tensor_tensor(out=x2, in0=y, in1=_bc(ri2, 2, D), op=A.mult)
    nc.vector.tensor_tensor(out=x2, in0=x2, in1=_bc(n2w_sb, 1, NT), op=A.mult)

    # weights: w1/w2 [256, 64] -> transposed [64, 256]; w3 [64, 256] -> [256, 64]
    w1n = consts.tile([128, 2, D], FP)
    w2n = consts.tile([128, 2, D], FP)
    w3n = consts.tile([64, DH], FP)
    nc.sync.dma_start(out=w1n, in_=w1_d.ap().rearrange("(c j) d -> j c d", j=128))
    nc.sync.dma_start(out=w2n, in_=w2_d.ap().rearrange("(c j) d -> j c d", j=128))
    nc.sync.dma_start(out=w3n, in_=w3_d.ap())
    w1T = consts.tile([64, DH], FR)   # [d, j]
    w2T = consts.tile([64, DH], FR)
    w3T = consts.tile([128, 2, D], FP)  # [j128, c, e]
    for c in range(2):
        t_ps = ps_tr.tile([64, 128], FP, tag="tr")
        nc.tensor.transpose(t_ps, w1n[:, c, :], ident)
        nc.scalar.copy(out=w1T[:, 128 * c:128 * (c + 1)], in_=t_ps)
        t_ps2 = ps_tr.tile([64, 128], FP, tag="tr")
        nc.tensor.transpose(t_ps2, w2n[:, c, :], ident)
        nc.scalar.copy(out=w2T[:, 128 * c:128 * (c + 1)], in_=t_ps2)
        t_ps3 = ps_tr.tile([128, 64], FP, tag="tr")
        nc.tensor.transpose(t_ps3, w3n[:, 128 * c:128 * (c + 1)],
                            ident[0:64, 0:64])
        nc.scalar.copy(out=w3T[:, c, :], in_=t_ps3)
    b1_sb = consts.tile([128, 2], FP)
    b2_sb = consts.tile([128, 2], FP)
    nc.sync.dma_start(out=b1_sb, in_=b1_d.ap().rearrange("(c j) -> j c", j=128))
    nc.sync.dma_start(out=b2_sb, in_=b2_d.ap().rearrange("(c j) -> j c", j=128))
    b3_sb = consts.tile([128, D], FP)
    nc.sync.dma_start(out=b3_sb,
                      in_=bass.AP(tensor=b3_d, offset=0, ap=[[0, 128], [1, D]]))

    # x2T [d, n] via PE transposes
    x2T = mlpw.tile([64, NT, 128], FR)
    for t in range(NT):
        xt_ps = ps_tr.tile([64, 128], FP, tag="tr")
        nc.tensor.transpose(xt_ps, x2[:, t, :], ident)
        nc.scalar.copy(out=x2T[:, t, :], in_=xt_ps)

    # y3 = y + b3 (residual with bias folded)
    nc.vector.tensor_tensor(out=y, in0=y, in1=_bc(b3_sb, 1, NT), op=A.add)

    # h1T/h2T [j, n] = w @ x2T; silu/bias per-partition
    hhT = mlpw.tile([128, 2, NT, 128], FP)   # [j128, c, nt, n]
    ps_m = ctx.enter_context(tc.tile_pool(name="ps_m", bufs=1, space="PSUM"))
    for c in range(2):
        h1_ps = ps_m.tile([128, NT, 128], FP, tag="h1")
        h2_ps = ps_m.tile([128, NT, 128], FP, tag="h2")
        for half in range(2):
            sl = slice(4 * half, 4 * (half + 1))
            nc.tensor.matmul(h1_ps[:, sl, :].rearrange("j t n -> j (t n)"),
                             lhsT=_fr(w1T[:, 128 * c:128 * (c + 1)]),
                             rhs=_fr(x2T[:, sl, :].rearrange("d t n -> d (t n)")),
                             start=True, stop=True)
            nc.tensor.matmul(h2_ps[:, sl, :].rearrange("j t n -> j (t n)"),
                             lhsT=_fr(w2T[:, 128 * c:128 * (c + 1)]),
                             rhs=_fr(x2T[:, sl, :].rearrange("d t n -> d (t n)")),
                             start=True, stop=True)
        h1b = mlpw.tile([128, NT, 128], FP, tag="h1b")
        nc.vector.tensor_scalar(h1b, h1_ps, b1_sb[:, c:c + 1], None, op0=A.add)
        sgm = mlpw.tile([128, NT, 128], FP, tag="sgm")
        nc.scalar.activation(out=sgm, in_=h1b, func=AF.Sigmoid)
        h2s = mlpw.tile([128, NT, 128], FP, tag="h2s")
        nc.vector.tensor_scalar(h2s, h2_ps, b2_sb[:, c:c + 1], None, op0=A.add)
        nc.vector.tensor_tensor(out=h1b, in0=h1b, in1=sgm, op=A.mult)
        nc.vector.tensor_tensor(out=hhT[:, c], in0=h1b, in1=h2s, op=A.mult)

    final = mlpw.tile([128, NT, D], FP)
    for t in range(NT):
        o_ps = ps_m.tile([128, D], FP, tag="o")
        for c in range(2):
            nc.tensor.matmul(o_ps, lhsT=hhT[:, c, t, :], rhs=w3T[:, c, :],
                             start=(c == 0), stop=(c == 1))
        nc.vector.scalar_tensor_tensor(out=final[:, t], in0=o_ps, scalar=1.0,
                                       in1=y[:, t], op0=A.mult, op1=A.add)

    nc.sync.dma_start(out=out_d.ap().rearrange("(t p) d -> p t d", p=128),
                      in_=final)


def build_program():
    from contextlib import ExitStack
    nc = bacc.Bacc("TRN2", target_bir_lowering=False, debug=False,
                   num_devices=NCORES)
    x_d = nc.dram_tensor("x", [N, D], FP, kind="ExternalInput")
    pos_d = nc.dram_tensor("pos", [N, D], FP, kind="ExternalInput")
    n1w_d = nc.dram_tensor("norm1_w", [D], FP, kind="ExternalInput")
    n2w_d = nc.dram_tensor("norm2_w", [D], FP, kind="ExternalInput")
    w1_d = nc.dram_tensor("w1_w", [DH, D], FP, kind="ExternalInput")
    b1_d = nc.dram_tensor("w1_b", [DH], FP, kind="ExternalInput")
    w2_d = nc.dram_tensor("w2_w", [DH, D], FP, kind="ExternalInput")
    b2_d = nc.dram_tensor("w2_b", [DH], FP, kind="ExternalInput")
    w3_d = nc.dram_tensor("w3_w", [D, DH], FP, kind="ExternalInput")
    b3_d = nc.dram_tensor("w3_b", [D], FP, kind="ExternalInput")
    out_d = nc.dram_tensor("out", [NPC, D], FP, kind="ExternalOutput")
    xr_dram = nc.dram_tensor("xr_own", [NPC, D], FP)

    tensors = (x_d, pos_d, n1w_d, n2w_d, w1_d, b1_d, w2_d, b2_d, w3_d, b3_d,
               out_d, xr_dram)
    with tile.TileContext(nc) as tc:
        with ExitStack() as ctx:
            build_kernel_body(nc, tc, ctx, tensors)
    nc.compile()
    return nc


_NC_CACHE = None


def _get_nc():
    global _NC_CACHE
    if _NC_CACHE is None:
        _NC_CACHE = build_program()
    return _NC_CACHE


def make_in_maps(inputs):
    in_maps = []
    for c in range(NCORES):
        sh = c * NPC
        m = {
            "x": np.ascontiguousarray(np.roll(inputs["x"], -sh, axis=0)),
            "pos": np.ascontiguousarray(np.roll(inputs["pos"], -sh, axis=0)),
            "norm1_w": inputs["norm1_w"], "norm2_w": inputs["norm2_w"],
            "w1_w": inputs["w1_w"], "w1_b": inputs["w1_b"],
            "w2_w": inputs["w2_w"], "w2_b": inputs["w2_b"],
            "w3_w": inputs["w3_w"], "w3_b": inputs["w3_b"],
        }
        in_maps.append({k: np.asarray(v, dtype=np.float32) for k, v in m.items()})
    return in_maps


def run_on_hw(inputs, trace=False):
    from concourse.bass_utils import run_bass_kernel_spmd
    nc = _get_nc()
    res = run_bass_kernel_spmd(nc, make_in_maps(inputs), list(range(NCORES)),
                               trace=trace)
    out = np.concatenate([res.results[c]["out"] for c in range(NCORES)], axis=0)
    return out, res


def kernel(**inputs):
    out, _ = run_on_hw(inputs, trace=False)
    return out

